# revision 1
# baseline (speedup 1.0000x reference)
"""DGCNN (nn_DGCNN_50594714747409) Bass/TRN2 kernel — 8-core data parallel.

Contract: kernel(**inputs) takes the FULL unsharded inputs (as produced by
setup_inputs()) and returns the FULL [16, 101] output. Internally shards the
batch (16) across 8 NeuronCores (2 samples/core), runs one SPMD Bass program
per core via bass_utils.run_bass_kernel_spmd, and concatenates the outputs.

Algorithm notes (exactness-preserving refactor of the reference):
  * EdgeConv: max_k(lrelu(bn(W @ [x_j - x_i; x_i]))) with bn scale > 0 and
    lrelu monotone ==> lrelu(bn(max_k(Wn@x_j) + (Wc-Wn)@x_i)). BN is folded
    into the conv weights on the host.
  * kNN: top-20 of s[i,j] = x_i.x_j - 0.5||x_j||^2 (same per-row order as the
    reference's -||x_i - x_j||^2). Computed on the tensor engine with an
    augmented matmul; exact top-20 per row via 3 rounds of the DVE's
    max8/max_index/match_replace instructions.
  * Neighbor feature max: per-point gather of 20 rows of a^T (in HBM) with a
    single 2560-descriptor SWDGE dma_gather per 128-point tile, then a strided
    DVE max-reduce over the 20 gathered rows.
  * conv5 + global max/mean pool fused on PSUM eviction; FC head batched over
    both samples per core.
"""

import numpy as np

import concourse.bass as bass
import concourse.bacc as bacc
import concourse.mybir as mybir
from concourse.tile import TileContext
from concourse import bass_utils

F32 = mybir.dt.float32
U32 = mybir.dt.uint32
I16 = mybir.dt.int16
ALU = mybir.AluOpType
ACTF = mybir.ActivationFunctionType

N = 1024
KNN = 20
NEG = -1e30
EPS = 1e-5
LAYERS = [(3, 64), (64, 64), (64, 128), (128, 256)]  # (C_in, O)
NCORES = 8
BPC = 2  # samples per core


def build_nc(bpc=BPC):
    nc = bacc.Bacc("TRN2", target_bir_lowering=False, debug=False)

    # ---- I/O ----
    x_in = nc.dram_tensor("x", [bpc, 3, N], F32, kind="ExternalInput")
    win = {}
    for l, (C, O) in enumerate(LAYERS, 1):
        win[f"wnt{l}"] = nc.dram_tensor(f"wnt{l}", [C, O], F32, kind="ExternalInput")
        win[f"wbt{l}"] = nc.dram_tensor(f"wbt{l}", [C, O], F32, kind="ExternalInput")
        win[f"br{l}"] = nc.dram_tensor(f"br{l}", [1, O], F32, kind="ExternalInput")
    win["w5t"] = nc.dram_tensor("w5t", [4, 128, N], F32, kind="ExternalInput")
    win["b5c"] = nc.dram_tensor("b5c", [128, 8], F32, kind="ExternalInput")
    win["w6t"] = nc.dram_tensor("w6t", [16, 128, 512], F32, kind="ExternalInput")
    win["b6r"] = nc.dram_tensor("b6r", [1, 512], F32, kind="ExternalInput")
    win["w7t"] = nc.dram_tensor("w7t", [4, 128, 256], F32, kind="ExternalInput")
    win["b7r"] = nc.dram_tensor("b7r", [1, 256], F32, kind="ExternalInput")
    win["w8t"] = nc.dram_tensor("w8t", [2, 128, 101], F32, kind="ExternalInput")
    win["b8r"] = nc.dram_tensor("b8r", [1, 101], F32, kind="ExternalInput")
    ident_in = nc.dram_tensor("ident", [128, 128], F32, kind="ExternalInput")
    ones_in = nc.dram_tensor("ones", [1, 128], F32, kind="ExternalInput")
    mhalf_in = nc.dram_tensor("mhalf", [128, 1], F32, kind="ExternalInput")
    out = nc.dram_tensor("out", [bpc, 101], F32, kind="ExternalOutput")

    with TileContext(nc) as tc:
        import contextlib
        ctx = contextlib.ExitStack()
        with ctx:
            wpool = ctx.enter_context(tc.tile_pool(name="w", bufs=1))
            pool = ctx.enter_context(tc.tile_pool(name="sb", bufs=2))
            big = ctx.enter_context(tc.tile_pool(name="big", bufs=1))
            psum = ctx.enter_context(tc.tile_pool(name="ps", bufs=2, space="PSUM"))
            dram = ctx.enter_context(tc.tile_pool(name="dr", bufs=2, space="DRAM"))

            # ---- stage weights/constants into SBUF ----
            ident = wpool.tile([128, 128], F32, tag="ident")
            nc.sync.dma_start(ident[:], ident_in[:])
            ones = wpool.tile([1, 128], F32, tag="ones")
            nc.sync.dma_start(ones[:], ones_in[:])
            mhalf = wpool.tile([128, 1], F32, tag="mhalf")
            nc.sync.dma_start(mhalf[:], mhalf_in[:])
            wsb = {}
            for l, (C, O) in enumerate(LAYERS, 1):
                wsb[f"wnt{l}"] = wpool.tile([C, O], F32, tag=f"wnt{l}", name=f"wnt{l}")
                wsb[f"wbt{l}"] = wpool.tile([C, O], F32, tag=f"wbt{l}", name=f"wbt{l}")
                wsb[f"br{l}"] = wpool.tile([1, O], F32, tag=f"br{l}", name=f"br{l}")
                for k in (f"wnt{l}", f"wbt{l}", f"br{l}"):
                    nc.sync.dma_start(wsb[k][:], win[k][:])
            for k, shp in [("w5t", [128, 4, N]), ("b5c", [128, 8]),
                           ("b6r", [1, 512]),
                           ("w7t", [128, 4, 256]), ("b7r", [1, 256]),
                           ("w8t", [128, 2, 101]), ("b8r", [1, 101])]:
                wsb[k] = wpool.tile(shp, F32, tag=k, name=k)
                if len(shp) == 3:
                    nc.sync.dma_start(wsb[k][:], win[k][:].rearrange("a b c -> b a c"))
                else:
                    nc.sync.dma_start(wsb[k][:], win[k][:])

            # per-sample persistent feature tiles (xc = concat of layer outputs)
            # xc[s][0]: ch 0-127 (x1 | x2), xc[s][1]: x3, xc[s][2:4]: x4
            xc = [[big.tile([128, N], F32, tag=f"xc{s}_{t}", name=f"xc{s}_{t}")
                   for t in range(4)] for s in range(bpc)]
            p2 = big.tile([128, 16, bpc], F32, tag="p2")  # pooled [max|mean]

            def edge_layer(s, l, cur, C, O, dests):
                """cur: AP [C, N] input features. dests[h]: list of
                (tile, row_off) for 128-row chunk h of the [O, N] output."""
                wnt, wbt, br = wsb[f"wnt{l}"], wsb[f"wbt{l}"], wsb[f"br{l}"]
                sq = pool.tile([C, N], F32, tag="sq", bufs=1)
                nc.scalar.activation(sq[:], cur, ACTF.Square)
                negxx = pool.tile([1, N], F32, tag="negxx")
                for jc in range(2):
                    xx_ps = psum.tile([1, 512], F32, tag="aux")
                    nc.tensor.matmul(xx_ps[:], mhalf[:C, :],
                                     sq[:, jc * 512:(jc + 1) * 512],
                                     start=True, stop=True)
                    nc.scalar.copy(negxx[:, jc * 512:(jc + 1) * 512], xx_ps[:])
                idx = pool.tile([128, 8, 24], U32, tag="idx")
                at_dr = dram.tile([N, O], F32, tag="at_dr")
                b2T = pool.tile([128, 8, O], F32, tag="b2T", bufs=1)
                for it in range(8):
                    isl = slice(it * 128, (it + 1) * 128)
                    d_ps = psum.tile([128, N], F32, tag="dist")
                    for jc in range(2):
                        jsl = slice(jc * 512, (jc + 1) * 512)
                        nc.tensor.matmul(d_ps[:, jsl], cur[:, isl], cur[:, jsl],
                                         start=True, stop=False)
                        nc.tensor.matmul(d_ps[:, jsl], ones[:, :128],
                                         negxx[:, jsl], start=False, stop=True)
                    dsb = pool.tile([128, N], F32, tag="dsb")
                    nc.scalar.copy(dsb[:, 0:512], d_ps[:, 0:512])
                    nc.scalar.copy(dsb[:, 512:N], d_ps[:, 512:N])
                    # exact top-20 per row: 3 rounds of max8
                    for r in range(3):
                        mx = pool.tile([128, 8], F32, tag="mx")
                        nc.vector.max(mx[:], dsb[:])
                        nc.vector.max_index(idx[:, it, r * 8:(r + 1) * 8],
                                            mx[:], dsb[:])
                        if r < 2:
                            nc.vector.match_replace(dsb[:], mx[:], dsb[:], NEG)
                    # aT (to HBM for the gather) / b2T n-tiles
                    a_ps = psum.tile([128, O], F32, tag="aux")
                    nc.tensor.matmul(a_ps[:], cur[:, isl], wnt[:],
                                     start=True, stop=True)
                    a_st = pool.tile([128, O], F32, tag="a_st")
                    nc.scalar.copy(a_st[:], a_ps[:])
                    nc.sync.dma_start(at_dr[isl, :], a_st[:])
                    b_ps = psum.tile([128, O], F32, tag="aux")
                    nc.tensor.matmul(b_ps[:], cur[:, isl], wbt[:],
                                     start=True, stop=False)
                    nc.tensor.matmul(b_ps[:], ones[:, :128], br[:],
                                     start=False, stop=True)
                    nc.scalar.copy(b2T[:, it, :], b_ps[:])
                # J wrap for dma_gather: jA[r, it*160 + t*8 + g] = idx[16g+r, it, t]
                # (PE transpose moves point-partition -> free; permuted engine
                #  copy makes the g axis contiguous; 2 clean DMAs finish it.)
                jA = dram.tile([16, 1280], I16, tag="jA")
                jAv = jA[:].rearrange("r (it t g) -> r it t g", it=8, t=KNN, g=8)
                for it in range(8):
                    idxf = pool.tile([128, KNN], F32, tag="idxf")
                    nc.vector.tensor_copy(idxf[:], idx[:, it, 0:KNN])
                    it_ps = psum.tile([KNN, 128], F32, tag="aux")
                    nc.tensor.transpose(it_ps[:], idxf[:], ident[:])
                    idxw = pool.tile([KNN, 128], I16, tag="idxw")
                    wv = idxw[:].rearrange("t (r g) -> t r g", r=16, g=8)
                    sv = it_ps[:].rearrange("t (g r) -> t r g", g=8, r=16)
                    nc.vector.tensor_copy(wv, sv)
                    nc.sync.dma_start(
                        jAv[:, it, :, :].rearrange("r t g -> t r g"), idxw[:])
                jsb = pool.tile([128, 1280], I16, tag="jsb")
                for gg in range(8):
                    nc.sync.dma_start(jsb[16 * gg:16 * (gg + 1), :], jA[:])
                # gather + 20-way max reduce per i-tile
                z = pool.tile([128, 8, O], F32, tag="z", bufs=1)
                for it in range(8):
                    jslice = jsb[:, it * 160:(it + 1) * 160]
                    if O <= 128:
                        g_t = pool.tile([128, KNN, O], F32, tag="gath")
                        nc.gpsimd.dma_gather(
                            out_ap=g_t[:], in_ap=at_dr[:], idxs_ap=jslice,
                            num_idxs=KNN * 128, num_idxs_reg=KNN * 128,
                            elem_size=O, single_packet=False)
                        nc.vector.tensor_reduce(
                            z[:, it, :], g_t[:].rearrange("p t o -> p o t"),
                            axis=mybir.AxisListType.X, op=ALU.max)
                    else:
                        for h in range(2):
                            g_t = pool.tile([128, KNN, 128], F32, tag="gath")
                            nc.gpsimd.dma_gather(
                                out_ap=g_t[:],
                                in_ap=at_dr[:, h * 128:(h + 1) * 128],
                                idxs_ap=jslice,
                                num_idxs=KNN * 128, num_idxs_reg=KNN * 128,
                                elem_size=128, elem_step=O, single_packet=False)
                            nc.vector.tensor_reduce(
                                z[:, it, h * 128:(h + 1) * 128],
                                g_t[:].rearrange("p t o -> p o t"),
                                axis=mybir.AxisListType.X, op=ALU.max)
                # z = lrelu(z + b2T)
                zf = z[:].rearrange("p a b -> p (a b)")
                bf = b2T[:].rearrange("p a b -> p (a b)")
                nc.vector.tensor_tensor(out=zf, in0=zf, in1=bf, op=ALU.add)
                nc.vector.scalar_tensor_tensor(out=zf, in0=zf, scalar=0.2,
                                               in1=zf, op0=ALU.mult, op1=ALU.max)
                # transpose [N, O] -> [O, N] dest rows
                for it in range(8):
                    for h in range((O + 127) // 128):
                        oc = min(128, O - h * 128)
                        t_ps = psum.tile([128, 128], F32, tag="aux")
                        nc.tensor.transpose(
                            t_ps[:oc, :], z[:, it, h * 128:h * 128 + oc], ident[:])
                        for dt_, roff in dests[h]:
                            nc.scalar.copy(
                                dt_[roff:roff + oc, it * 128:(it + 1) * 128],
                                t_ps[:oc, :])

            for s in range(bpc):
                x_sb = pool.tile([3, N], F32, tag="x_in")
                nc.sync.dma_start(x_sb[:], x_in[s])
                x2own = pool.tile([64, N], F32, tag="x2own", bufs=1)
                edge_layer(s, 1, x_sb[:], 3, 64, [[(xc[s][0], 0)]])
                edge_layer(s, 2, xc[s][0][0:64, :], 64, 64,
                           [[(xc[s][0], 64), (x2own, 0)]])
                edge_layer(s, 3, x2own[:], 64, 128, [[(xc[s][1], 0)]])
                edge_layer(s, 4, xc[s][1][:], 128, 256,
                           [[(xc[s][2], 0)], [(xc[s][3], 0)]])

                # conv5 + fused global max/mean pool
                accs = pool.tile([128, 8, 2], F32, tag="accs")
                pmax = pool.tile([128, 8, 2], F32, tag="pmax")
                for m in range(8):
                    for c in range(2):
                        y_ps = psum.tile([128, 512], F32, tag="dist")
                        for kt in range(4):
                            nc.tensor.matmul(
                                y_ps[:], wsb["w5t"][:, kt, m * 128:(m + 1) * 128],
                                xc[s][kt][:, c * 512:(c + 1) * 512],
                                start=(kt == 0), stop=(kt == 3))
                        z5 = pool.tile([128, 512], F32, tag="z5")
                        nc.scalar.activation(z5[:], y_ps[:], ACTF.Identity,
                                             bias=wsb["b5c"][:, m:m + 1])
                        scr = pool.tile([128, 512], F32, tag="scr5")
                        nc.vector.scalar_tensor_tensor(
                            out=scr[:], in0=z5[:], scalar=0.2, in1=z5[:],
                            op0=ALU.mult, op1=ALU.max,
                            accum_out=accs[:, m, c:c + 1])
                        nc.vector.tensor_reduce(
                            pmax[:, m, c:c + 1], z5[:],
                            axis=mybir.AxisListType.X, op=ALU.max)
                pm = pool.tile([128, 8], F32, tag="pm")
                nc.vector.tensor_tensor(pm[:], pmax[:, :, 0], pmax[:, :, 1],
                                        op=ALU.max)
                nc.vector.scalar_tensor_tensor(
                    out=p2[:, 0:8, s], in0=pm[:], scalar=0.2, in1=pm[:],
                    op0=ALU.mult, op1=ALU.max)
                asm = pool.tile([128, 8], F32, tag="asm")
                nc.vector.tensor_tensor(asm[:], accs[:, :, 0], accs[:, :, 1],
                                        op=ALU.add)
                nc.vector.tensor_scalar(
                    out=p2[:, 8:16, s], in0=asm[:], scalar1=1.0 / N,
                    scalar2=None, op0=ALU.mult)

            # ---- FC head (both samples batched) ----
            h1_ps = psum.tile([bpc, 512], F32, tag="fc")
            for t in range(16):
                w6kt = pool.tile([128, 512], F32, tag="w6kt")
                nc.sync.dma_start(w6kt[:], win["w6t"][t])
                nc.tensor.matmul(h1_ps[:], p2[:, t, :], w6kt[:],
                                 start=(t == 0), stop=False)
            nc.tensor.matmul(h1_ps[:], ones[:, 0:bpc], wsb["b6r"][:],
                             start=False, stop=True)
            h1 = pool.tile([bpc, 512], F32, tag="h1")
            nc.scalar.copy(h1[:], h1_ps[:])
            nc.vector.scalar_tensor_tensor(out=h1[:], in0=h1[:], scalar=0.2,
                                           in1=h1[:], op0=ALU.mult, op1=ALU.max)
            h1t = pool.tile([128, 4, bpc], F32, tag="h1t")
            for kt in range(4):
                t_ps = psum.tile([128, bpc], F32, tag="aux")
                nc.tensor.transpose(t_ps[:], h1[:, kt * 128:(kt + 1) * 128],
                                    ident[0:bpc, 0:bpc])
                nc.scalar.copy(h1t[:, kt, :], t_ps[:])
            h2_ps = psum.tile([bpc, 256], F32, tag="fc")
            for t in range(4):
                nc.tensor.matmul(h2_ps[:], h1t[:, t, :], wsb["w7t"][:, t, :],
                                 start=(t == 0), stop=False)
            nc.tensor.matmul(h2_ps[:], ones[:, 0:bpc], wsb["b7r"][:],
                             start=False, stop=True)
            h2 = pool.tile([bpc, 256], F32, tag="h2")
            nc.scalar.copy(h2[:], h2_ps[:])
            nc.vector.scalar_tensor_tensor(out=h2[:], in0=h2[:], scalar=0.2,
                                           in1=h2[:], op0=ALU.mult, op1=ALU.max)
            h2t = pool.tile([128, 2, bpc], F32, tag="h2t")
            for kt in range(2):
                t_ps = psum.tile([128, bpc], F32, tag="aux")
                nc.tensor.transpose(t_ps[:], h2[:, kt * 128:(kt + 1) * 128],
                                    ident[0:bpc, 0:bpc])
                nc.scalar.copy(h2t[:, kt, :], t_ps[:])
            o_ps = psum.tile([bpc, 101], F32, tag="fc")
            for t in range(2):
                nc.tensor.matmul(o_ps[:], h2t[:, t, :], wsb["w8t"][:, t, 0:101],
                                 start=(t == 0), stop=False)
            nc.tensor.matmul(o_ps[:], ones[:, 0:bpc], wsb["b8r"][:],
                             start=False, stop=True)
            o_sb = pool.tile([bpc, 101], F32, tag="osb")
            nc.scalar.copy(o_sb[:], o_ps[:])
            nc.sync.dma_start(out[:], o_sb[:])

    nc.finalize()
    return nc


def prep_weights(inp):
    """Host-side: fold BN into weights; device-friendly layouts."""
    d = {}
    f32 = np.float32
    for l, (C, O) in enumerate(LAYERS, 1):
        w = np.asarray(inp[f"w{l}"], f32)
        g, b = np.asarray(inp[f"g{l}"], f32), np.asarray(inp[f"b{l}"], f32)
        m, v = np.asarray(inp[f"m{l}"], f32), np.asarray(inp[f"v{l}"], f32)
        s = g / np.sqrt(v + EPS)
        assert (s > 0).all(), "BN scale must be positive for the max/act swap"
        wn = w[:, :C] * s[:, None]
        wb = (w[:, C:] - w[:, :C]) * s[:, None]
        d[f"wnt{l}"] = np.ascontiguousarray(wn.T, f32)
        d[f"wbt{l}"] = np.ascontiguousarray(wb.T, f32)
        d[f"br{l}"] = np.ascontiguousarray((b - m * s)[None, :], f32)
    s5 = np.asarray(inp["g5"], f32) / np.sqrt(np.asarray(inp["v5"], f32) + EPS)
    assert (s5 > 0).all()
    d["w5t"] = np.ascontiguousarray(
        (np.asarray(inp["w5"], f32) * s5[:, None]).T.reshape(4, 128, N), f32)
    d["b5c"] = np.ascontiguousarray(
        (np.asarray(inp["b5"], f32) - np.asarray(inp["m5"], f32) * s5)
        .reshape(8, 128).T, f32)
    s6 = np.asarray(inp["g6"], f32) / np.sqrt(np.asarray(inp["v6"], f32) + EPS)
    d["w6t"] = np.ascontiguousarray(
        (np.asarray(inp["wl1"], f32) * s6[:, None]).T.reshape(16, 128, 512), f32)
    d["b6r"] = np.ascontiguousarray(
        (np.asarray(inp["b6"], f32) - np.asarray(inp["m6"], f32) * s6)[None, :], f32)
    s7 = np.asarray(inp["g7"], f32) / np.sqrt(np.asarray(inp["v7"], f32) + EPS)
    d["w7t"] = np.ascontiguousarray(
        (np.asarray(inp["wl2"], f32) * s7[:, None]).T.reshape(4, 128, 256), f32)
    d["b7r"] = np.ascontiguousarray(
        (s7 * (np.asarray(inp["bl2"], f32) - np.asarray(inp["m7"], f32))
         + np.asarray(inp["b7"], f32))[None, :], f32)
    d["w8t"] = np.ascontiguousarray(
        np.asarray(inp["wl3"], f32).T.reshape(2, 128, 101), f32)
    d["b8r"] = np.ascontiguousarray(np.asarray(inp["bl3"], f32)[None, :], f32)
    d["ident"] = np.eye(128, dtype=f32)
    d["ones"] = np.ones((1, 128), f32)
    d["mhalf"] = np.full((128, 1), -0.5, f32)
    return d


_CACHE = {}


def _get_nc():
    if "nc" not in _CACHE:
        _CACHE["nc"] = build_nc()
    return _CACHE["nc"]


def kernel(**inputs):
    x = np.ascontiguousarray(np.asarray(inputs["x"], np.float32))
    assert x.shape == (16, 3, N), x.shape
    prep = prep_weights(inputs)
    nc = _get_nc()
    in_maps = []
    for c in range(NCORES):
        m = dict(prep)
        m["x"] = np.ascontiguousarray(x[c * BPC:(c + 1) * BPC])
        in_maps.append(m)
    res = bass_utils.run_bass_kernel_spmd(nc, in_maps, core_ids=list(range(NCORES)))
    out = np.concatenate([r["out"] for r in res.results], axis=0)
    return out.astype(np.float32)



# revision 20
# speedup vs baseline: 1.3935x; 1.3935x over previous
"""DGCNN (nn_DGCNN_50594714747409) Bass/TRN2 kernel — 8-core data parallel.

Contract: kernel(**inputs) takes the FULL unsharded inputs (as produced by
setup_inputs()) and returns the FULL [16, 101] output. Internally shards the
batch (16) across 8 NeuronCores (2 samples/core), runs one SPMD Bass program
per core via bass_utils.run_bass_kernel_spmd, and concatenates the outputs.

Algorithm notes (exactness-preserving refactor of the reference):
  * EdgeConv: max_k(lrelu(bn(W @ [x_j - x_i; x_i]))) with bn scale > 0 and
    lrelu monotone ==> lrelu(bn(max_k(Wn@x_j) + (Wc-Wn)@x_i)). BN is folded
    into the conv weights on the host.
  * kNN: top-20 of s[i,j] = x_i.x_j - 0.5||x_j||^2 (same per-row order as the
    reference's -||x_i - x_j||^2). The -0.5||x_j||^2 column bias rides as an
    extra contraction row (features augmented with a ones row) so the dist
    matmul needs no separate bias matmul (layers 1-3; layer 4 has C=128 and
    keeps the ones-matmul bias). Exact top-20 per row via 3 rounds of the
    DVE's max8/max_index/match_replace instructions.
  * a = Wn@x_j and b = (Wc-Wn)@x_i + br are computed in ONE matmul against a
    fused [wn|wb] weight block (bias via the same ones row).
  * Neighbor feature max: per-point gather of 20 rows of a^T (in HBM) with a
    single 2560-descriptor SWDGE dma_gather per 128-point tile, then a
    strided DVE max-reduce over the 20 gathered rows. Layer 4 gathers the
    full 256-channel rows in bf16 (one gather instead of two f32 halves —
    SWDGE descriptor generation is the kernel bottleneck at ~7.5ns/desc).
  * The two samples per core are software-pipelined: sample A's SWDGE gather
    stream overlaps sample B's dist/topk compute, keeping the Pool engine
    (gathers) busy ~continuously.
  * conv5 + global max/mean pool fused on PSUM eviction; FC head batched
    over both samples per core.
"""

import numpy as np

import concourse.bass as bass
import concourse.bacc as bacc
import concourse.mybir as mybir
from concourse.tile import TileContext
from concourse import bass_utils

F32 = mybir.dt.float32
BF16 = mybir.dt.bfloat16
U32 = mybir.dt.uint32
I16 = mybir.dt.int16
ALU = mybir.AluOpType
ACTF = mybir.ActivationFunctionType

N = 1024
KNN = 20
NEG = -1e30
EPS = 1e-5
LAYERS = [(3, 64), (64, 64), (64, 128), (128, 256)]  # (C_in, O)
NCORES = 8
BPC = 2  # samples per core


def build_nc(bpc=BPC):
    nc = bacc.Bacc("TRN2", target_bir_lowering=False, debug=False)

    # ---- I/O ----
    x_in = nc.dram_tensor("x", [bpc, 3, N], F32, kind="ExternalInput")
    win = {}
    for l, (C, O) in enumerate(LAYERS, 1):
        rows = C + 1 if l < 4 else C
        win[f"wab{l}"] = nc.dram_tensor(f"wab{l}", [rows, 2 * O], F32,
                                        kind="ExternalInput")
    win["brz4"] = nc.dram_tensor("brz4", [1, 512], F32, kind="ExternalInput")
    win["w5t"] = nc.dram_tensor("w5t", [4, 128, N], F32, kind="ExternalInput")
    win["b5c"] = nc.dram_tensor("b5c", [128, 8], F32, kind="ExternalInput")
    win["w6t"] = nc.dram_tensor("w6t", [16, 128, 512], F32, kind="ExternalInput")
    win["b6r"] = nc.dram_tensor("b6r", [1, 512], F32, kind="ExternalInput")
    win["w7t"] = nc.dram_tensor("w7t", [4, 128, 256], F32, kind="ExternalInput")
    win["b7r"] = nc.dram_tensor("b7r", [1, 256], F32, kind="ExternalInput")
    win["w8t"] = nc.dram_tensor("w8t", [2, 128, 101], F32, kind="ExternalInput")
    win["b8r"] = nc.dram_tensor("b8r", [1, 101], F32, kind="ExternalInput")
    ident_in = nc.dram_tensor("ident", [128, 128], F32, kind="ExternalInput")
    ones_in = nc.dram_tensor("ones", [1, 128], F32, kind="ExternalInput")
    mhalf_in = nc.dram_tensor("mhalf", [128, 1], F32, kind="ExternalInput")
    out = nc.dram_tensor("out", [bpc, 101], F32, kind="ExternalOutput")

    with TileContext(nc) as tc:
        import contextlib
        ctx = contextlib.ExitStack()
        with ctx:
            wpool = ctx.enter_context(tc.tile_pool(name="w", bufs=1))
            pool = ctx.enter_context(tc.tile_pool(name="sb", bufs=2))
            big = ctx.enter_context(tc.tile_pool(name="big", bufs=1))
            psum = ctx.enter_context(tc.tile_pool(name="ps", bufs=2, space="PSUM"))
            dram = ctx.enter_context(tc.tile_pool(name="dr", bufs=2, space="DRAM"))

            # ---- stage weights/constants into SBUF ----
            ident = wpool.tile([128, 128], F32, tag="ident")
            nc.sync.dma_start(ident[:], ident_in[:])
            ones = wpool.tile([1, 128], F32, tag="ones")
            nc.sync.dma_start(ones[:], ones_in[:])
            mhalf = wpool.tile([128, 1], F32, tag="mhalf")
            nc.sync.dma_start(mhalf[:], mhalf_in[:])
            wsb = {}
            for l, (C, O) in enumerate(LAYERS, 1):
                rows = C + 1 if l < 4 else C
                wsb[f"wab{l}"] = wpool.tile([rows, 2 * O], F32, tag=f"wab{l}",
                                            name=f"wab{l}")
                nc.sync.dma_start(wsb[f"wab{l}"][:], win[f"wab{l}"][:])
            for k, shp in [("brz4", [1, 512]), ("w5t", [128, 4, N]),
                           ("b5c", [128, 8]), ("b6r", [1, 512]),
                           ("w7t", [128, 4, 256]), ("b7r", [1, 256]),
                           ("w8t", [128, 2, 101]), ("b8r", [1, 101])]:
                wsb[k] = wpool.tile(shp, F32, tag=k, name=k)
                if len(shp) == 3:
                    nc.sync.dma_start(wsb[k][:], win[k][:].rearrange("a b c -> b a c"))
                else:
                    nc.sync.dma_start(wsb[k][:], win[k][:])

            # per-sample persistent feature tiles (xc = concat of layer outputs)
            # xc[s][0]: ch 0-127 (x1 | x2), xc[s][1]: x3, xc[s][2:4]: x4
            xc = [[big.tile([128, N], F32, tag=f"xc{s}_{t}", name=f"xc{s}_{t}")
                   for t in range(4)] for s in range(bpc)]
            # augmented working tiles: rows 0..C-1 features, row C = -0.5||x||^2
            wk0 = [big.tile([4, N], F32, tag=f"wk0_{s}", name=f"wk0_{s}")
                   for s in range(bpc)]
            wk1 = [big.tile([65, N], F32, tag=f"wk1_{s}", name=f"wk1_{s}")
                   for s in range(bpc)]
            wk2 = [big.tile([65, N], F32, tag=f"wk2_{s}", name=f"wk2_{s}")
                   for s in range(bpc)]

            p2 = big.tile([128, 16, bpc], F32, tag="p2")  # pooled [max|mean]

            def phase1(s, l, C, O, ft, st_out):
                """dist + topk + fused a|b matmuls + index assembly for
                (sample s, layer l). Generator: yields after each i-tile so
                the scheduler can interleave it with the other sample's
                gather/reduce stream at i-tile granularity. ft: [C(+1), N]
                features; for aug layers (l<4) row C gets -0.5||x_j||^2.
                Fills st_out (state for phase2)."""
                aug = l < 4
                wab = wsb[f"wab{l}"]
                sq = pool.tile([C, N], F32, tag=f"sq{s}", bufs=1)
                nc.scalar.activation(sq[:], ft[0:C, :], ACTF.Square)
                # engine ops need quad-aligned base partitions; rows at
                # partition C (3/64) are written via DMA from a staging tile
                nxst = pool.tile([1, N], F32, tag=f"nxs{s}", bufs=1)
                for jc in range(2):
                    jsl = slice(jc * 512, (jc + 1) * 512)
                    xx_ps = psum.tile([1, 512], F32, tag=f"aux")
                    nc.tensor.matmul(xx_ps[:], mhalf[:C, :], sq[:, jsl],
                                     start=True, stop=True)
                    nc.scalar.copy(nxst[:, jsl], xx_ps[:])
                if aug:
                    nc.sync.dma_start(ft[C:C + 1, :], nxst[:])
                    lhsa = pool.tile([C + 1, 128], F32, tag=f"lh{s}", bufs=1)
                    nc.sync.dma_start(lhsa[C:C + 1, :], ones_in[:])
                idx = pool.tile([128, 8, 24], U32, tag=f"idx{s}", bufs=1)
                at_dtype = BF16 if l == 4 else F32
                at_dr = dram.tile([N, O], at_dtype, tag=f"at{s}")
                b2T = pool.tile([128, 8, O], F32, tag=f"b2_{s}", bufs=1)
                jA = dram.tile([16, 1280], I16, tag=f"jA{s}")
                jAv = jA[:].rearrange("r (it t g) -> r it t g", it=8, t=KNN, g=8)
                for it in range(8):
                    isl = slice(it * 128, (it + 1) * 128)
                    if aug:
                        nc.scalar.copy(lhsa[0:C, :], ft[0:C, isl])
                        lhs_ap = lhsa[:]
                    else:
                        lhs_ap = ft[0:C, isl]
                    dsb = pool.tile([128, N], F32, tag=f"dsb{s}", bufs=2)
                    for jc in range(2):
                        jsl = slice(jc * 512, (jc + 1) * 512)
                        d_ps = psum.tile([128, 512], F32, tag=f"d{s}", bufs=2)
                        rhs = ft[0:C + 1, jsl] if aug else ft[0:C, jsl]
                        nc.tensor.matmul(d_ps[:], lhs_ap, rhs,
                                         start=True, stop=aug)
                        if not aug:
                            nc.tensor.matmul(d_ps[:], ones[:, :128],
                                             nxst[:, jsl], start=False, stop=True)
                        nc.scalar.copy(dsb[:, jsl], d_ps[:])
                    # exact top-20 per row: 3 rounds of max8
                    for r in range(3):
                        mx = pool.tile([128, 8], F32, tag=f"mx{s}")
                        nc.vector.max(mx[:], dsb[:])
                        nc.vector.max_index(idx[:, it, r * 8:(r + 1) * 8],
                                            mx[:], dsb[:])
                        if r < 2:
                            nc.vector.match_replace(dsb[:], mx[:], dsb[:], NEG)
                    # fused a|b matmul: cols 0:O = Wn@x (+0), O:2O = Wb@x + br
                    ab_ps = psum.tile([128, 2 * O], F32, tag=f"ab{s}", bufs=1)
                    nc.tensor.matmul(ab_ps[:], lhs_ap, wab[:],
                                     start=True, stop=aug)
                    if not aug:
                        nc.tensor.matmul(ab_ps[:], ones[:, :128], wsb["brz4"][:],
                                         start=False, stop=True)
                    a_st = pool.tile([128, O], at_dtype, tag=f"as{s}", bufs=2)
                    nc.scalar.copy(a_st[:], ab_ps[:, 0:O])
                    nc.sync.dma_start(at_dr[isl, :], a_st[:])
                    nc.scalar.copy(b2T[:, it, :], ab_ps[:, O:2 * O])
                    # index assembly for the gather (per i-tile)
                    idxf = pool.tile([128, KNN], F32, tag=f"ixf{s}")
                    nc.vector.tensor_copy(idxf[:], idx[:, it, 0:KNN])
                    it_ps = psum.tile([KNN, 128], F32, tag=f"aux")
                    nc.tensor.transpose(it_ps[:], idxf[:], ident[:])
                    idxw = pool.tile([KNN, 128], I16, tag=f"ixw{s}")
                    wv = idxw[:].rearrange("t (r g) -> t r g", r=16, g=8)
                    sv = it_ps[:].rearrange("t (g r) -> t r g", g=8, r=16)
                    nc.vector.tensor_copy(wv, sv)
                    nc.sync.dma_start(
                        jAv[:, it, :, :].rearrange("r t g -> t r g"), idxw[:])
                    yield
                jsb = pool.tile([128, 1280], I16, tag=f"js{s}", bufs=1)
                for gg in range(8):
                    nc.sync.dma_start(jsb[16 * gg:16 * (gg + 1), :], jA[:])
                st_out.update(at=at_dr, jsb=jsb, b2T=b2T, O=O, l=l)

            def phase2(s, st, dests):
                """gather + 20-way max + bias + lrelu + transpose to dests.
                Generator: yields after each i-tile's gather+reduce."""
                O, l = st["O"], st["l"]
                at_dr, jsb, b2T = st["at"], st["jsb"], st["b2T"]
                gt_dtype = BF16 if l == 4 else F32
                z = pool.tile([128, 8, O], F32, tag=f"z{s}", bufs=1)
                for it in range(8):
                    if l < 3:
                        # one 2560-descriptor gather of full 20-neighbor rows
                        jslice = jsb[:, it * 160:(it + 1) * 160]
                        g_t = pool.tile([128, KNN, O], gt_dtype, tag=f"g{s}",
                                        bufs=2)
                        nc.gpsimd.dma_gather(
                            out_ap=g_t[:], in_ap=at_dr[:], idxs_ap=jslice,
                            num_idxs=KNN * 128, num_idxs_reg=KNN * 128,
                            elem_size=O, single_packet=False)
                        nc.vector.tensor_reduce(
                            z[:, it, :], g_t[:].rearrange("p t o -> p o t"),
                            axis=mybir.AxisListType.X, op=ALU.max)
                    else:
                        # full O-channel rows, split by neighbor halves
                        # (jA packs pos = t*8+g, so t-halves are contiguous)
                        ztmp = pool.tile([128, O], F32, tag=f"zt{s}", bufs=1)
                        for hf in range(2):
                            jslice = jsb[:, it * 160 + hf * 80:
                                         it * 160 + (hf + 1) * 80]
                            g_t = pool.tile([128, KNN // 2, O], gt_dtype,
                                            tag=f"g{s}", bufs=2)
                            nc.gpsimd.dma_gather(
                                out_ap=g_t[:], in_ap=at_dr[:], idxs_ap=jslice,
                                num_idxs=KNN * 64, num_idxs_reg=KNN * 64,
                                elem_size=O, single_packet=False)
                            nc.vector.tensor_reduce(
                                ztmp[:] if hf else z[:, it, :],
                                g_t[:].rearrange("p t o -> p o t"),
                                axis=mybir.AxisListType.X, op=ALU.max)
                        nc.vector.tensor_tensor(z[:, it, :], z[:, it, :],
                                                ztmp[:], op=ALU.max)
                    yield
                # z = lrelu(z + b2T)
                zf = z[:].rearrange("p a b -> p (a b)")
                bf = b2T[:].rearrange("p a b -> p (a b)")
                nc.vector.tensor_tensor(out=zf, in0=zf, in1=bf, op=ALU.add)
                nc.vector.scalar_tensor_tensor(out=zf, in0=zf, scalar=0.2,
                                               in1=zf, op0=ALU.mult, op1=ALU.max)
                # transpose [N, O] -> [O, N] dest rows
                for it in range(8):
                    for h in range((O + 127) // 128):
                        oc = min(128, O - h * 128)
                        t_ps = psum.tile([128, 128], F32, tag=f"aux")
                        nc.tensor.transpose(
                            t_ps[:oc, :], z[:, it, h * 128:h * 128 + oc], ident[:])
                        for dt_, roff in dests[h]:
                            nc.scalar.copy(
                                dt_[roff:roff + oc, it * 128:(it + 1) * 128],
                                t_ps[:oc, :])

            def conv5(s):
                # conv5 + fused global max/mean pool (generator: yield per m)
                accs = pool.tile([128, 8, 2], F32, tag=f"accs{s}", bufs=1)
                pmax = pool.tile([128, 8, 2], F32, tag=f"pmax{s}", bufs=1)
                for m in range(8):
                    for c in range(2):
                        y_ps = psum.tile([128, 512], F32, tag=f"d{s}", bufs=2)
                        for kt in range(4):
                            nc.tensor.matmul(
                                y_ps[:], wsb["w5t"][:, kt, m * 128:(m + 1) * 128],
                                xc[s][kt][:, c * 512:(c + 1) * 512],
                                start=(kt == 0), stop=(kt == 3))
                        z5 = pool.tile([128, 512], F32, tag=f"z5{s}", bufs=1)
                        nc.scalar.activation(z5[:], y_ps[:], ACTF.Identity,
                                             bias=wsb["b5c"][:, m:m + 1])
                        scr = pool.tile([128, 512], F32, tag=f"scr{s}", bufs=1)
                        nc.vector.scalar_tensor_tensor(
                            out=scr[:], in0=z5[:], scalar=0.2, in1=z5[:],
                            op0=ALU.mult, op1=ALU.max,
                            accum_out=accs[:, m, c:c + 1])
                        nc.vector.tensor_reduce(
                            pmax[:, m, c:c + 1], z5[:],
                            axis=mybir.AxisListType.X, op=ALU.max)
                    yield
                pm = pool.tile([128, 8], F32, tag=f"pm{s}")
                nc.vector.tensor_tensor(pm[:], pmax[:, :, 0], pmax[:, :, 1],
                                        op=ALU.max)
                nc.vector.scalar_tensor_tensor(
                    out=p2[:, 0:8, s], in0=pm[:], scalar=0.2, in1=pm[:],
                    op0=ALU.mult, op1=ALU.max)
                asm = pool.tile([128, 8], F32, tag=f"asm{s}")
                nc.vector.tensor_tensor(asm[:], accs[:, :, 0], accs[:, :, 1],
                                        op=ALU.add)
                nc.vector.tensor_scalar(
                    out=p2[:, 8:16, s], in0=asm[:], scalar1=1.0 / N,
                    scalar2=None, op0=ALU.mult)

            # ---- load inputs ----
            for s in range(bpc):
                nc.sync.dma_start(wk0[s][0:3, :], x_in[s])

            # layer input tiles / dest lists per (s, layer)
            fts = [[wk0[s], wk1[s], wk2[s], xc[s][1]] for s in range(bpc)]
            dsts = [[[[(xc[s][0], 0), (wk1[s], 0)]],
                     [[(xc[s][0], 64), (wk2[s], 0)]],
                     [[(xc[s][1], 0)]],
                     [[(xc[s][2], 0)], [(xc[s][3], 0)]]] for s in range(bpc)]

            # ---- software-pipelined schedule across the two samples ----
            # Each pair interleaves (at i-tile granularity) sample s's
            # dist/topk (PE+DVE) with sample s''s gather/reduce (Pool+DVE),
            # keeping the SWDGE gather stream — the bottleneck — busy.
            # Phase1 steps are emitted first each iteration so topk never
            # queues behind a gather-blocked reduce in the DVE stream.
            st = [dict() for _ in range(bpc)]

            def mk1(s, l):
                C, O = LAYERS[l - 1]
                return phase1(s, l, C, O, fts[s][l - 1][:], st[s])

            def mk2(s, l):
                return phase2(s, st[s], dsts[s][l - 1])

            def drive(g):
                for _ in g:
                    pass

            def pair(first, second):
                da = db = False
                while not (da and db):
                    if not da:
                        try:
                            next(first)
                        except StopIteration:
                            da = True
                    if not db:
                        try:
                            next(second)
                        except StopIteration:
                            db = True

            drive(mk1(0, 1))
            pair(mk1(1, 1), mk2(0, 1))
            pair(mk1(0, 2), mk2(1, 1))
            pair(mk1(1, 2), mk2(0, 2))
            pair(mk1(0, 3), mk2(1, 2))
            pair(mk1(1, 3), mk2(0, 3))
            pair(mk1(0, 4), mk2(1, 3))
            pair(mk1(1, 4), mk2(0, 4))
            pair(conv5(0), mk2(1, 4))
            drive(conv5(1))

            # ---- FC head (both samples batched) ----
            h1_ps = psum.tile([bpc, 512], F32, tag="ab0", bufs=1)
            for t in range(16):
                w6kt = pool.tile([128, 512], F32, tag="w6kt")
                nc.sync.dma_start(w6kt[:], win["w6t"][t])
                nc.tensor.matmul(h1_ps[:], p2[:, t, :], w6kt[:],
                                 start=(t == 0), stop=False)
            nc.tensor.matmul(h1_ps[:], ones[:, 0:bpc], wsb["b6r"][:],
                             start=False, stop=True)
            h1 = pool.tile([bpc, 512], F32, tag="h1")
            nc.scalar.copy(h1[:], h1_ps[:])
            nc.vector.scalar_tensor_tensor(out=h1[:], in0=h1[:], scalar=0.2,
                                           in1=h1[:], op0=ALU.mult, op1=ALU.max)
            h1t = pool.tile([128, 4, bpc], F32, tag="h1t")
            for kt in range(4):
                t_ps = psum.tile([128, bpc], F32, tag="aux")
                nc.tensor.transpose(t_ps[:], h1[:, kt * 128:(kt + 1) * 128],
                                    ident[0:bpc, 0:bpc])
                nc.scalar.copy(h1t[:, kt, :], t_ps[:])
            h2_ps = psum.tile([bpc, 256], F32, tag="ab1", bufs=1)
            for t in range(4):
                nc.tensor.matmul(h2_ps[:], h1t[:, t, :], wsb["w7t"][:, t, :],
                                 start=(t == 0), stop=False)
            nc.tensor.matmul(h2_ps[:], ones[:, 0:bpc], wsb["b7r"][:],
                             start=False, stop=True)
            h2 = pool.tile([bpc, 256], F32, tag="h2")
            nc.scalar.copy(h2[:], h2_ps[:])
            nc.vector.scalar_tensor_tensor(out=h2[:], in0=h2[:], scalar=0.2,
                                           in1=h2[:], op0=ALU.mult, op1=ALU.max)
            h2t = pool.tile([128, 2, bpc], F32, tag="h2t")
            for kt in range(2):
                t_ps = psum.tile([128, bpc], F32, tag="aux")
                nc.tensor.transpose(t_ps[:], h2[:, kt * 128:(kt + 1) * 128],
                                    ident[0:bpc, 0:bpc])
                nc.scalar.copy(h2t[:, kt, :], t_ps[:])
            o_ps = psum.tile([bpc, 101], F32, tag="ab0", bufs=1)
            for t in range(2):
                nc.tensor.matmul(o_ps[:], h2t[:, t, :], wsb["w8t"][:, t, 0:101],
                                 start=(t == 0), stop=False)
            nc.tensor.matmul(o_ps[:], ones[:, 0:bpc], wsb["b8r"][:],
                             start=False, stop=True)
            o_sb = pool.tile([bpc, 101], F32, tag="osb")
            nc.scalar.copy(o_sb[:], o_ps[:])
            nc.sync.dma_start(out[:], o_sb[:])

    nc.finalize()
    return nc


def prep_weights(inp):
    """Host-side: fold BN into weights; device-friendly layouts."""
    d = {}
    f32 = np.float32
    for l, (C, O) in enumerate(LAYERS, 1):
        w = np.asarray(inp[f"w{l}"], f32)
        g, b = np.asarray(inp[f"g{l}"], f32), np.asarray(inp[f"b{l}"], f32)
        m, v = np.asarray(inp[f"m{l}"], f32), np.asarray(inp[f"v{l}"], f32)
        s = g / np.sqrt(v + EPS)
        assert (s > 0).all(), "BN scale must be positive for the max/act swap"
        wn = (w[:, :C] * s[:, None]).T          # [C, O]
        wb = ((w[:, C:] - w[:, :C]) * s[:, None]).T
        br = b - m * s
        if l < 4:
            wab = np.zeros((C + 1, 2 * O), f32)
            wab[:C, :O] = wn
            wab[:C, O:] = wb
            wab[C, O:] = br
        else:
            wab = np.ascontiguousarray(np.concatenate([wn, wb], axis=1), f32)
            brz = np.zeros((1, 512), f32)
            brz[0, 256:] = br
            d["brz4"] = brz
        d[f"wab{l}"] = np.ascontiguousarray(wab, f32)
    s5 = np.asarray(inp["g5"], f32) / np.sqrt(np.asarray(inp["v5"], f32) + EPS)
    assert (s5 > 0).all()
    d["w5t"] = np.ascontiguousarray(
        (np.asarray(inp["w5"], f32) * s5[:, None]).T.reshape(4, 128, N), f32)
    d["b5c"] = np.ascontiguousarray(
        (np.asarray(inp["b5"], f32) - np.asarray(inp["m5"], f32) * s5)
        .reshape(8, 128).T, f32)
    s6 = np.asarray(inp["g6"], f32) / np.sqrt(np.asarray(inp["v6"], f32) + EPS)
    d["w6t"] = np.ascontiguousarray(
        (np.asarray(inp["wl1"], f32) * s6[:, None]).T.reshape(16, 128, 512), f32)
    d["b6r"] = np.ascontiguousarray(
        (np.asarray(inp["b6"], f32) - np.asarray(inp["m6"], f32) * s6)[None, :], f32)
    s7 = np.asarray(inp["g7"], f32) / np.sqrt(np.asarray(inp["v7"], f32) + EPS)
    d["w7t"] = np.ascontiguousarray(
        (np.asarray(inp["wl2"], f32) * s7[:, None]).T.reshape(4, 128, 256), f32)
    d["b7r"] = np.ascontiguousarray(
        (s7 * (np.asarray(inp["bl2"], f32) - np.asarray(inp["m7"], f32))
         + np.asarray(inp["b7"], f32))[None, :], f32)
    d["w8t"] = np.ascontiguousarray(
        np.asarray(inp["wl3"], f32).T.reshape(2, 128, 101), f32)
    d["b8r"] = np.ascontiguousarray(np.asarray(inp["bl3"], f32)[None, :], f32)
    d["ident"] = np.eye(128, dtype=f32)
    d["ones"] = np.ones((1, 128), f32)
    d["mhalf"] = np.full((128, 1), -0.5, f32)
    return d


_CACHE = {}


def _get_nc():
    if "nc" not in _CACHE:
        _CACHE["nc"] = build_nc()
    return _CACHE["nc"]


def kernel(**inputs):
    x = np.ascontiguousarray(np.asarray(inputs["x"], np.float32))
    assert x.shape == (16, 3, N), x.shape
    prep = prep_weights(inputs)
    nc = _get_nc()
    in_maps = []
    for c in range(NCORES):
        m = dict(prep)
        m["x"] = np.ascontiguousarray(x[c * BPC:(c + 1) * BPC])
        in_maps.append(m)
    res = bass_utils.run_bass_kernel_spmd(nc, in_maps, core_ids=list(range(NCORES)))
    out = np.concatenate([r["out"] for r in res.results], axis=0)
    return out.astype(np.float32)


# revision 24
# speedup vs baseline: 1.7446x; 1.2519x over previous
"""DGCNN (nn_DGCNN_50594714747409) Bass/TRN2 kernel — 8-core data parallel.

Contract: kernel(**inputs) takes the FULL unsharded inputs (as produced by
setup_inputs()) and returns the FULL [16, 101] output. Internally shards the
batch (16) across 8 NeuronCores (2 samples/core), runs one SPMD Bass program
per core via bass_utils.run_bass_kernel_spmd, and concatenates the outputs.

Algorithm notes (exactness-preserving refactor of the reference):
  * EdgeConv: max_k(lrelu(bn(W @ [x_j - x_i; x_i]))) with bn scale > 0 and
    lrelu monotone ==> lrelu(bn(max_k(Wn@x_j) + (Wc-Wn)@x_i)). BN is folded
    into the conv weights on the host.
  * kNN: top-20 of s[i,j] = x_i.x_j - 0.5||x_j||^2 (same per-row order as the
    reference's -||x_i - x_j||^2). The -0.5||x_j||^2 column bias rides as an
    extra contraction row (features augmented with a ones row) so the dist
    matmul needs no separate bias matmul (layers 1-3; layer 4 has C=128 and
    keeps the ones-matmul bias). Exact top-20 per row via 3 rounds of the
    DVE's max8/max_index/match_replace instructions.
  * a = Wn@x_j and b = (Wc-Wn)@x_i + br are computed in ONE matmul against a
    fused [wn|wb] weight block (bias via the same ones row).
  * Neighbor feature max: per-point gather of 20 rows of a^T (in HBM) with a
    single 2560-descriptor SWDGE dma_gather per 128-point tile, then a
    strided DVE max-reduce over the 20 gathered rows. Layer 4 gathers the
    full 256-channel rows in bf16 (one gather instead of two f32 halves —
    SWDGE descriptor generation is the kernel bottleneck at ~7.5ns/desc).
  * The two samples per core are software-pipelined: sample A's SWDGE gather
    stream overlaps sample B's dist/topk compute, keeping the Pool engine
    (gathers) busy ~continuously.
  * conv5 + global max/mean pool fused on PSUM eviction; FC head batched
    over both samples per core.
"""

import numpy as np

import concourse.bass as bass
import concourse.bacc as bacc
import concourse.mybir as mybir
from concourse.tile import TileContext, add_dep_helper
from concourse import bass_utils

F32 = mybir.dt.float32
BF16 = mybir.dt.bfloat16
U32 = mybir.dt.uint32
I16 = mybir.dt.int16
ALU = mybir.AluOpType
ACTF = mybir.ActivationFunctionType

N = 1024
KNN = 20
NEG = -1e30
EPS = 1e-5
LAYERS = [(3, 64), (64, 64), (64, 128), (128, 256)]  # (C_in, O)
NCORES = 8
BPC = 2  # samples per core


def build_nc(bpc=BPC):
    nc = bacc.Bacc("TRN2", target_bir_lowering=False, debug=False)

    # ---- I/O ----
    x_in = nc.dram_tensor("x", [bpc, 3, N], F32, kind="ExternalInput")
    win = {}
    for l, (C, O) in enumerate(LAYERS, 1):
        rows = C + 1 if l < 4 else C
        win[f"wab{l}"] = nc.dram_tensor(f"wab{l}", [rows, 2 * O], F32,
                                        kind="ExternalInput")
    win["brz4"] = nc.dram_tensor("brz4", [1, 512], F32, kind="ExternalInput")
    win["w5t"] = nc.dram_tensor("w5t", [4, 128, N], F32, kind="ExternalInput")
    win["b5c"] = nc.dram_tensor("b5c", [128, 8], F32, kind="ExternalInput")
    win["w6t"] = nc.dram_tensor("w6t", [16, 128, 512], F32, kind="ExternalInput")
    win["b6r"] = nc.dram_tensor("b6r", [1, 512], F32, kind="ExternalInput")
    win["w7t"] = nc.dram_tensor("w7t", [4, 128, 256], F32, kind="ExternalInput")
    win["b7r"] = nc.dram_tensor("b7r", [1, 256], F32, kind="ExternalInput")
    win["w8t"] = nc.dram_tensor("w8t", [2, 128, 101], F32, kind="ExternalInput")
    win["b8r"] = nc.dram_tensor("b8r", [1, 101], F32, kind="ExternalInput")
    ident_in = nc.dram_tensor("ident", [128, 128], F32, kind="ExternalInput")
    ones_in = nc.dram_tensor("ones", [1, 128], F32, kind="ExternalInput")
    mhalf_in = nc.dram_tensor("mhalf", [128, 1], F32, kind="ExternalInput")
    out = nc.dram_tensor("out", [bpc, 101], F32, kind="ExternalOutput")

    with TileContext(nc) as tc:
        import contextlib
        ctx = contextlib.ExitStack()
        with ctx:
            wpool = ctx.enter_context(tc.tile_pool(name="w", bufs=1))
            pool = ctx.enter_context(tc.tile_pool(name="sb", bufs=2))
            big = ctx.enter_context(tc.tile_pool(name="big", bufs=1))
            psum = ctx.enter_context(tc.tile_pool(name="ps", bufs=2, space="PSUM"))
            dram = ctx.enter_context(tc.tile_pool(name="dr", bufs=2, space="DRAM"))

            # ---- stage weights/constants into SBUF ----
            ident = wpool.tile([128, 128], F32, tag="ident")
            nc.sync.dma_start(ident[:], ident_in[:])
            ones = wpool.tile([1, 128], F32, tag="ones")
            nc.sync.dma_start(ones[:], ones_in[:])
            mhalf = wpool.tile([128, 1], F32, tag="mhalf")
            nc.sync.dma_start(mhalf[:], mhalf_in[:])
            wsb = {}
            for l, (C, O) in enumerate(LAYERS, 1):
                rows = C + 1 if l < 4 else C
                wsb[f"wab{l}"] = wpool.tile([rows, 2 * O], F32, tag=f"wab{l}",
                                            name=f"wab{l}")
                nc.sync.dma_start(wsb[f"wab{l}"][:], win[f"wab{l}"][:])
            for k, shp in [("brz4", [1, 512]), ("w5t", [128, 4, N]),
                           ("b5c", [128, 8]), ("b6r", [1, 512]),
                           ("w7t", [128, 4, 256]), ("b7r", [1, 256]),
                           ("w8t", [128, 2, 101]), ("b8r", [1, 101])]:
                wsb[k] = wpool.tile(shp, F32, tag=k, name=k)
                if len(shp) == 3:
                    nc.sync.dma_start(wsb[k][:], win[k][:].rearrange("a b c -> b a c"))
                else:
                    nc.sync.dma_start(wsb[k][:], win[k][:])

            # per-sample persistent feature tiles (xc = concat of layer outputs)
            # xc[s][0]: ch 0-127 (x1 | x2), xc[s][1]: x3, xc[s][2:4]: x4
            xc = [[big.tile([128, N], F32, tag=f"xc{s}_{t}", name=f"xc{s}_{t}")
                   for t in range(4)] for s in range(bpc)]
            # augmented working tiles: rows 0..C-1 features, row C = -0.5||x||^2
            wk0 = [big.tile([4, N], F32, tag=f"wk0_{s}", name=f"wk0_{s}")
                   for s in range(bpc)]
            wk1 = [big.tile([65, N], F32, tag=f"wk1_{s}", name=f"wk1_{s}")
                   for s in range(bpc)]
            wk2 = [big.tile([65, N], F32, tag=f"wk2_{s}", name=f"wk2_{s}")
                   for s in range(bpc)]

            p2 = big.tile([128, 16, bpc], F32, tag="p2")  # pooled [max|mean]
            # cross-generator channel: last topk inst of the in-flight
            # phase1 i-tile (see phase2.gate)
            dep_chan = {"d": None}

            def phase1(s, l, C, O, ft, st_out):
                """dist + topk + fused a|b matmuls + index assembly for
                (sample s, layer l). Generator: yields after each i-tile so
                the scheduler can interleave it with the other sample's
                gather/reduce stream at i-tile granularity. ft: [C(+1), N]
                features; for aug layers (l<4) row C gets -0.5||x_j||^2.
                Fills st_out (state for phase2)."""
                aug = l < 4
                wab = wsb[f"wab{l}"]
                sq = pool.tile([C, N], F32, tag=f"sq{s}", bufs=1)
                nc.scalar.activation(sq[:], ft[0:C, :], ACTF.Square)
                # engine ops need quad-aligned base partitions; rows at
                # partition C (3/64) are written via DMA from a staging tile
                nxst = pool.tile([1, N], F32, tag=f"nxs{s}", bufs=1)
                for jc in range(2):
                    jsl = slice(jc * 512, (jc + 1) * 512)
                    xx_ps = psum.tile([1, 512], F32, tag=f"aux")
                    nc.tensor.matmul(xx_ps[:], mhalf[:C, :], sq[:, jsl],
                                     start=True, stop=True)
                    nc.scalar.copy(nxst[:, jsl], xx_ps[:])
                if aug:
                    nc.sync.dma_start(ft[C:C + 1, :], nxst[:])
                    lhsa = pool.tile([C + 1, 128], F32, tag=f"lh{s}", bufs=1)
                    nc.sync.dma_start(lhsa[C:C + 1, :], ones_in[:])
                idx = pool.tile([128, 8, 24], U32, tag=f"idx{s}", bufs=1)
                at_dtype = BF16 if l == 4 else F32
                at_dr = dram.tile([N, O], at_dtype, tag=f"at{s}")
                b2T = pool.tile([128, 8, O], F32, tag=f"b2_{s}", bufs=1)
                jA = dram.tile([16, 1280], I16, tag=f"jA{s}")
                jAv = jA[:].rearrange("r (it t g) -> r it t g", it=8, t=KNN, g=8)
                for it in range(8):
                    isl = slice(it * 128, (it + 1) * 128)
                    if aug:
                        nc.scalar.copy(lhsa[0:C, :], ft[0:C, isl])
                        lhs_ap = lhsa[:]
                    else:
                        lhs_ap = ft[0:C, isl]
                    dsb = pool.tile([128, N], F32, tag=f"dsb{s}", bufs=2)
                    for jc in range(2):
                        jsl = slice(jc * 512, (jc + 1) * 512)
                        d_ps = psum.tile([128, 512], F32, tag=f"d{s}", bufs=2)
                        rhs = ft[0:C + 1, jsl] if aug else ft[0:C, jsl]
                        nc.tensor.matmul(d_ps[:], lhs_ap, rhs,
                                         start=True, stop=aug)
                        if not aug:
                            nc.tensor.matmul(d_ps[:], ones[:, :128],
                                             nxst[:, jsl], start=False, stop=True)
                        nc.scalar.copy(dsb[:, jsl], d_ps[:])
                    # exact top-20 per row: 3 rounds of max8
                    for r in range(3):
                        mx = pool.tile([128, 8], F32, tag=f"mx{s}")
                        nc.vector.max(mx[:], dsb[:])
                        mi = nc.vector.max_index(idx[:, it, r * 8:(r + 1) * 8],
                                                 mx[:], dsb[:])
                        if r < 2:
                            nc.vector.match_replace(dsb[:], mx[:], dsb[:], NEG)
                    # publish this i-tile's last topk inst: the paired
                    # phase2 gates its reduces on it so the scheduler's
                    # (gather-optimistic) sim can't queue reduces ahead of
                    # topk in the DVE stream (a reduce at the DVE head
                    # blocks ~16us on its gather)
                    dep_chan["d"] = mi.ins
                    # fused a|b matmul: cols 0:O = Wn@x (+0), O:2O = Wb@x + br
                    ab_ps = psum.tile([128, 2 * O], F32, tag=f"ab{s}", bufs=1)
                    nc.tensor.matmul(ab_ps[:], lhs_ap, wab[:],
                                     start=True, stop=aug)
                    if not aug:
                        nc.tensor.matmul(ab_ps[:], ones[:, :128], wsb["brz4"][:],
                                         start=False, stop=True)
                    a_st = pool.tile([128, O], at_dtype, tag=f"as{s}", bufs=2)
                    nc.scalar.copy(a_st[:], ab_ps[:, 0:O])
                    nc.sync.dma_start(at_dr[isl, :], a_st[:])
                    nc.scalar.copy(b2T[:, it, :], ab_ps[:, O:2 * O])
                    # index assembly for the gather (per i-tile)
                    idxf = pool.tile([128, KNN], F32, tag=f"ixf{s}")
                    nc.vector.tensor_copy(idxf[:], idx[:, it, 0:KNN])
                    it_ps = psum.tile([KNN, 128], F32, tag=f"aux")
                    nc.tensor.transpose(it_ps[:], idxf[:], ident[:])
                    idxw = pool.tile([KNN, 128], I16, tag=f"ixw{s}")
                    wv = idxw[:].rearrange("t (r g) -> t r g", r=16, g=8)
                    sv = it_ps[:].rearrange("t (g r) -> t r g", g=8, r=16)
                    nc.vector.tensor_copy(wv, sv)
                    nc.sync.dma_start(
                        jAv[:, it, :, :].rearrange("r t g -> t r g"), idxw[:])
                    yield
                jsb = pool.tile([128, 1280], I16, tag=f"js{s}", bufs=1)
                for gg in range(8):
                    nc.sync.dma_start(jsb[16 * gg:16 * (gg + 1), :], jA[:])
                st_out.update(at=at_dr, jsb=jsb, b2T=b2T, O=O, l=l)

            def phase2(s, st, dests):
                """gather + 20-way max + bias + lrelu + transpose to dests.
                Generator: yields after each i-tile's gather+reduce; the
                (it-1) finalize rides with step it so its DVE ops only ever
                wait on an already-completed gather."""
                O, l = st["O"], st["l"]
                at_dr, jsb, b2T = st["at"], st["jsb"], st["b2T"]
                gt_dtype = BF16 if l == 4 else F32
                z = pool.tile([128, 8, O], F32, tag=f"z{s}", bufs=1)

                def gate(binst):
                    if dep_chan["d"] is not None:
                        add_dep_helper(binst.ins, dep_chan["d"],
                                       reason="reduce after paired topk")

                def finalize(it):
                    # z = lrelu(z + b2T); transpose [N, O] -> [O, N] dests
                    zi = z[:, it, :]
                    nc.vector.tensor_tensor(out=zi, in0=zi, in1=b2T[:, it, :],
                                            op=ALU.add)
                    nc.vector.scalar_tensor_tensor(
                        out=zi, in0=zi, scalar=0.2, in1=zi,
                        op0=ALU.mult, op1=ALU.max)
                    for h in range((O + 127) // 128):
                        oc = min(128, O - h * 128)
                        t_ps = psum.tile([128, 128], F32, tag=f"aux")
                        nc.tensor.transpose(
                            t_ps[:oc, :], z[:, it, h * 128:h * 128 + oc],
                            ident[:])
                        for dt_, roff in dests[h]:
                            nc.scalar.copy(
                                dt_[roff:roff + oc, it * 128:(it + 1) * 128],
                                t_ps[:oc, :])

                for it in range(8):
                    if l < 3:
                        # one 2560-descriptor gather of full 20-neighbor rows
                        jslice = jsb[:, it * 160:(it + 1) * 160]
                        g_t = pool.tile([128, KNN, O], gt_dtype, tag=f"g{s}",
                                        bufs=2)
                        nc.gpsimd.dma_gather(
                            out_ap=g_t[:], in_ap=at_dr[:], idxs_ap=jslice,
                            num_idxs=KNN * 128, num_idxs_reg=KNN * 128,
                            elem_size=O, single_packet=False)
                        gate(nc.vector.tensor_reduce(
                            z[:, it, :], g_t[:].rearrange("p t o -> p o t"),
                            axis=mybir.AxisListType.X, op=ALU.max))
                    else:
                        # full O-channel rows, split by neighbor halves
                        # (jA packs pos = t*8+g, so t-halves are contiguous)
                        ztmp = pool.tile([128, O], F32, tag=f"zt{s}", bufs=1)
                        for hf in range(2):
                            jslice = jsb[:, it * 160 + hf * 80:
                                         it * 160 + (hf + 1) * 80]
                            g_t = pool.tile([128, KNN // 2, O], gt_dtype,
                                            tag=f"g{s}", bufs=2)
                            nc.gpsimd.dma_gather(
                                out_ap=g_t[:], in_ap=at_dr[:], idxs_ap=jslice,
                                num_idxs=KNN * 64, num_idxs_reg=KNN * 64,
                                elem_size=O, single_packet=False)
                            gate(nc.vector.tensor_reduce(
                                ztmp[:] if hf else z[:, it, :],
                                g_t[:].rearrange("p t o -> p o t"),
                                axis=mybir.AxisListType.X, op=ALU.max))
                        nc.vector.tensor_tensor(z[:, it, :], z[:, it, :],
                                                ztmp[:], op=ALU.max)
                    if it:
                        finalize(it - 1)
                    yield
                finalize(7)

            def conv5(s):
                # conv5 + fused global max/mean pool (generator: yield per m)
                accs = pool.tile([128, 8, 2], F32, tag=f"accs{s}", bufs=1)
                pmax = pool.tile([128, 8, 2], F32, tag=f"pmax{s}", bufs=1)
                for m in range(8):
                    for c in range(2):
                        y_ps = psum.tile([128, 512], F32, tag=f"d{s}", bufs=2)
                        for kt in range(4):
                            nc.tensor.matmul(
                                y_ps[:], wsb["w5t"][:, kt, m * 128:(m + 1) * 128],
                                xc[s][kt][:, c * 512:(c + 1) * 512],
                                start=(kt == 0), stop=(kt == 3))
                        z5 = pool.tile([128, 512], F32, tag=f"z5{s}", bufs=1)
                        nc.scalar.activation(z5[:], y_ps[:], ACTF.Identity,
                                             bias=wsb["b5c"][:, m:m + 1])
                        scr = pool.tile([128, 512], F32, tag=f"scr{s}", bufs=1)
                        nc.vector.scalar_tensor_tensor(
                            out=scr[:], in0=z5[:], scalar=0.2, in1=z5[:],
                            op0=ALU.mult, op1=ALU.max,
                            accum_out=accs[:, m, c:c + 1])
                        nc.vector.tensor_reduce(
                            pmax[:, m, c:c + 1], z5[:],
                            axis=mybir.AxisListType.X, op=ALU.max)
                    yield
                pm = pool.tile([128, 8], F32, tag=f"pm{s}")
                nc.vector.tensor_tensor(pm[:], pmax[:, :, 0], pmax[:, :, 1],
                                        op=ALU.max)
                nc.vector.scalar_tensor_tensor(
                    out=p2[:, 0:8, s], in0=pm[:], scalar=0.2, in1=pm[:],
                    op0=ALU.mult, op1=ALU.max)
                asm = pool.tile([128, 8], F32, tag=f"asm{s}")
                nc.vector.tensor_tensor(asm[:], accs[:, :, 0], accs[:, :, 1],
                                        op=ALU.add)
                nc.vector.tensor_scalar(
                    out=p2[:, 8:16, s], in0=asm[:], scalar1=1.0 / N,
                    scalar2=None, op0=ALU.mult)

            # ---- load inputs ----
            for s in range(bpc):
                nc.sync.dma_start(wk0[s][0:3, :], x_in[s])

            # layer input tiles / dest lists per (s, layer)
            fts = [[wk0[s], wk1[s], wk2[s], xc[s][1]] for s in range(bpc)]
            dsts = [[[[(xc[s][0], 0), (wk1[s], 0)]],
                     [[(xc[s][0], 64), (wk2[s], 0)]],
                     [[(xc[s][1], 0)]],
                     [[(xc[s][2], 0)], [(xc[s][3], 0)]]] for s in range(bpc)]

            # ---- software-pipelined schedule across the two samples ----
            # Each pair interleaves (at i-tile granularity) sample s's
            # dist/topk (PE+DVE) with sample s''s gather/reduce (Pool+DVE),
            # keeping the SWDGE gather stream — the bottleneck — busy.
            # Phase1 steps are emitted first each iteration so topk never
            # queues behind a gather-blocked reduce in the DVE stream.
            st = [dict() for _ in range(bpc)]

            def mk1(s, l):
                C, O = LAYERS[l - 1]
                return phase1(s, l, C, O, fts[s][l - 1][:], st[s])

            def mk2(s, l):
                return phase2(s, st[s], dsts[s][l - 1])

            def drive(g):
                for _ in g:
                    pass

            def pair(first, second):
                da = db = False
                while not (da and db):
                    if not da:
                        try:
                            next(first)
                        except StopIteration:
                            da = True
                    if not db:
                        try:
                            next(second)
                        except StopIteration:
                            db = True

            drive(mk1(0, 1))
            pair(mk1(1, 1), mk2(0, 1))
            pair(mk1(0, 2), mk2(1, 1))
            pair(mk1(1, 2), mk2(0, 2))
            pair(mk1(0, 3), mk2(1, 2))
            pair(mk1(1, 3), mk2(0, 3))
            pair(mk1(0, 4), mk2(1, 3))
            pair(mk1(1, 4), mk2(0, 4))
            pair(conv5(0), mk2(1, 4))
            drive(conv5(1))

            # ---- FC head (both samples batched) ----
            h1_ps = psum.tile([bpc, 512], F32, tag="ab0", bufs=1)
            for t in range(16):
                w6kt = pool.tile([128, 512], F32, tag="w6kt")
                nc.sync.dma_start(w6kt[:], win["w6t"][t])
                nc.tensor.matmul(h1_ps[:], p2[:, t, :], w6kt[:],
                                 start=(t == 0), stop=False)
            nc.tensor.matmul(h1_ps[:], ones[:, 0:bpc], wsb["b6r"][:],
                             start=False, stop=True)
            h1 = pool.tile([bpc, 512], F32, tag="h1")
            nc.scalar.copy(h1[:], h1_ps[:])
            nc.vector.scalar_tensor_tensor(out=h1[:], in0=h1[:], scalar=0.2,
                                           in1=h1[:], op0=ALU.mult, op1=ALU.max)
            h1t = pool.tile([128, 4, bpc], F32, tag="h1t")
            for kt in range(4):
                t_ps = psum.tile([128, bpc], F32, tag="aux")
                nc.tensor.transpose(t_ps[:], h1[:, kt * 128:(kt + 1) * 128],
                                    ident[0:bpc, 0:bpc])
                nc.scalar.copy(h1t[:, kt, :], t_ps[:])
            h2_ps = psum.tile([bpc, 256], F32, tag="ab1", bufs=1)
            for t in range(4):
                nc.tensor.matmul(h2_ps[:], h1t[:, t, :], wsb["w7t"][:, t, :],
                                 start=(t == 0), stop=False)
            nc.tensor.matmul(h2_ps[:], ones[:, 0:bpc], wsb["b7r"][:],
                             start=False, stop=True)
            h2 = pool.tile([bpc, 256], F32, tag="h2")
            nc.scalar.copy(h2[:], h2_ps[:])
            nc.vector.scalar_tensor_tensor(out=h2[:], in0=h2[:], scalar=0.2,
                                           in1=h2[:], op0=ALU.mult, op1=ALU.max)
            h2t = pool.tile([128, 2, bpc], F32, tag="h2t")
            for kt in range(2):
                t_ps = psum.tile([128, bpc], F32, tag="aux")
                nc.tensor.transpose(t_ps[:], h2[:, kt * 128:(kt + 1) * 128],
                                    ident[0:bpc, 0:bpc])
                nc.scalar.copy(h2t[:, kt, :], t_ps[:])
            o_ps = psum.tile([bpc, 101], F32, tag="ab0", bufs=1)
            for t in range(2):
                nc.tensor.matmul(o_ps[:], h2t[:, t, :], wsb["w8t"][:, t, 0:101],
                                 start=(t == 0), stop=False)
            nc.tensor.matmul(o_ps[:], ones[:, 0:bpc], wsb["b8r"][:],
                             start=False, stop=True)
            o_sb = pool.tile([bpc, 101], F32, tag="osb")
            nc.scalar.copy(o_sb[:], o_ps[:])
            nc.sync.dma_start(out[:], o_sb[:])

    nc.finalize()
    return nc


def prep_weights(inp):
    """Host-side: fold BN into weights; device-friendly layouts."""
    d = {}
    f32 = np.float32
    for l, (C, O) in enumerate(LAYERS, 1):
        w = np.asarray(inp[f"w{l}"], f32)
        g, b = np.asarray(inp[f"g{l}"], f32), np.asarray(inp[f"b{l}"], f32)
        m, v = np.asarray(inp[f"m{l}"], f32), np.asarray(inp[f"v{l}"], f32)
        s = g / np.sqrt(v + EPS)
        assert (s > 0).all(), "BN scale must be positive for the max/act swap"
        wn = (w[:, :C] * s[:, None]).T          # [C, O]
        wb = ((w[:, C:] - w[:, :C]) * s[:, None]).T
        br = b - m * s
        if l < 4:
            wab = np.zeros((C + 1, 2 * O), f32)
            wab[:C, :O] = wn
            wab[:C, O:] = wb
            wab[C, O:] = br
        else:
            wab = np.ascontiguousarray(np.concatenate([wn, wb], axis=1), f32)
            brz = np.zeros((1, 512), f32)
            brz[0, 256:] = br
            d["brz4"] = brz
        d[f"wab{l}"] = np.ascontiguousarray(wab, f32)
    s5 = np.asarray(inp["g5"], f32) / np.sqrt(np.asarray(inp["v5"], f32) + EPS)
    assert (s5 > 0).all()
    d["w5t"] = np.ascontiguousarray(
        (np.asarray(inp["w5"], f32) * s5[:, None]).T.reshape(4, 128, N), f32)
    d["b5c"] = np.ascontiguousarray(
        (np.asarray(inp["b5"], f32) - np.asarray(inp["m5"], f32) * s5)
        .reshape(8, 128).T, f32)
    s6 = np.asarray(inp["g6"], f32) / np.sqrt(np.asarray(inp["v6"], f32) + EPS)
    d["w6t"] = np.ascontiguousarray(
        (np.asarray(inp["wl1"], f32) * s6[:, None]).T.reshape(16, 128, 512), f32)
    d["b6r"] = np.ascontiguousarray(
        (np.asarray(inp["b6"], f32) - np.asarray(inp["m6"], f32) * s6)[None, :], f32)
    s7 = np.asarray(inp["g7"], f32) / np.sqrt(np.asarray(inp["v7"], f32) + EPS)
    d["w7t"] = np.ascontiguousarray(
        (np.asarray(inp["wl2"], f32) * s7[:, None]).T.reshape(4, 128, 256), f32)
    d["b7r"] = np.ascontiguousarray(
        (s7 * (np.asarray(inp["bl2"], f32) - np.asarray(inp["m7"], f32))
         + np.asarray(inp["b7"], f32))[None, :], f32)
    d["w8t"] = np.ascontiguousarray(
        np.asarray(inp["wl3"], f32).T.reshape(2, 128, 101), f32)
    d["b8r"] = np.ascontiguousarray(np.asarray(inp["bl3"], f32)[None, :], f32)
    d["ident"] = np.eye(128, dtype=f32)
    d["ones"] = np.ones((1, 128), f32)
    d["mhalf"] = np.full((128, 1), -0.5, f32)
    return d


_CACHE = {}


def _get_nc():
    if "nc" not in _CACHE:
        _CACHE["nc"] = build_nc()
    return _CACHE["nc"]


def kernel(**inputs):
    x = np.ascontiguousarray(np.asarray(inputs["x"], np.float32))
    assert x.shape == (16, 3, N), x.shape
    prep = prep_weights(inputs)
    nc = _get_nc()
    in_maps = []
    for c in range(NCORES):
        m = dict(prep)
        m["x"] = np.ascontiguousarray(x[c * BPC:(c + 1) * BPC])
        in_maps.append(m)
    res = bass_utils.run_bass_kernel_spmd(nc, in_maps, core_ids=list(range(NCORES)))
    out = np.concatenate([r["out"] for r in res.results], axis=0)
    return out.astype(np.float32)
